# revision 43
# baseline (speedup 1.0000x reference)
"""Trainium2 kernel for nn_BernNet_47364899340878.

Math note (why the device kernel is just the MLP):
  The reference computes  out = sum_{j=0..K} c_j * relu(temp_j) * L^j (2I-L)^{K-j} h
  with c_j = C(K,j)/2^K and h = relu(x@W1+b1)@W2+b2.  The graded inputs pin
  temp = ones (spec fill "ones"), so relu(temp_j) = 1 for all j.  L and
  (2I - L) are commuting polynomials in the normalized adjacency, so the
  binomial theorem gives

      sum_j C(K,j) L^j (2I-L)^{K-j} = (L + 2I - L)^K = (2I)^K = 2^K I,

  i.e. the whole K=10 Bernstein propagation is exactly the identity map and
  out == h.  A non-ones temp (never the case for the graded inputs) falls
  back to a host implementation of the propagation for correctness.

Device kernel: h = relu(x@W1+b1), e = exp(h@W2 + b2), row-sharded over 8
NeuronCores (12500 rows each).  The device ships e (bf16) only; the host
finishes log_softmax exactly: out = ln(e), lse = ln(sum e), logp = out-lse.
The kernel is HBM-bandwidth bound; traffic per core is ~12.8 MB in +
~1.0 MB out.  Design notes (from perfetto/ntff analysis):
  - every input DMA uses ALL 128 SBUF partitions (HWDGE splits a transfer
    across SDMA engines by partition count; 128 -> all 16 engines).  The
    contraction is host-padded 500 -> 512 = 4 x 128 chunks,
  - ALL of x stays resident in SBUF (100KB of the 208KB/partition): 13
    persistent group tiles (one 500-row starter + 12 x 1000-row pairs), so
    no pool-recycle gating; groups are issued ~2 ahead of consumption —
    a free-running 425 GB/s burst costs PE clock (power throttle, HAM
    duty drops 8/8 -> 4/8) more than it buys,
  - per 500-row block: mm1 = 4 x 500-col matmuls accumulating h^T
    [128,500] in ONE PSUM bank; relu+bias on DVE -> bf16 [64,500]; mm2 =
    ONE matmul, stationary W2 [64,40], moving relu-h [64,500] -> logits^T
    [40,500] in PSUM (classes on partitions); exp on ACT reads PSUM and
    writes the bf16 output tile directly with bias=b2 (exact b2 handling),
  - the device runs NO reduce / Ln / subtract / copy: the host sums the
    SAME bf16 exp values in fp64 (lse error ~6e-4, far under the bf16
    matmul noise ~5e-3 vs the 2e-2 gate),
  - a memset scratch feeds ~14 PE warm-up matmuls + ACT/DVE warm-ups
    emitted before any data lands: starts the HAM clock ramp ~4us earlier
    and pre-loads the ACT function table off the critical path,
  - Exp and Relu are pinned to one ACT table set so the kernel does a
    single table load,
  - output quads [40, 4x500] bf16 ship via GpSimd SWDGE so their issue
    cost never delays ACT's exp queue.
"""

import numpy as np

_N = 100000
_FIN = 500
_FPAD = 512  # contraction padded to 4 chunks x 128 partitions
_HID = 64
_CLS = 40
_NCORES = 8
_RPC = _N // _NCORES  # 12500 rows per core
_P = 128  # contraction partitions per chunk
_KC = 4  # contraction chunks
_BLK = 500  # rows per block
_NBLK = _RPC // _BLK  # 25
_NSING = 2  # single-block groups (blocks 1,2) sized for the early HBM ramp
_NPAIR = 11  # paired 1000-row groups covering blocks 3..24
_NQUAD = 6  # output DMAs of 4 blocks each (blocks 0..23)

_CACHE = {}


def _build_bass():
    """Build the per-core Bass program (shared by all 8 cores)."""
    from contextlib import ExitStack

    import concourse.bacc as bacc
    import concourse.mybir as mybir
    import concourse.tile as tile

    fp32 = mybir.dt.float32
    bf16 = mybir.dt.bfloat16
    AF = mybir.ActivationFunctionType
    OP = mybir.AluOpType

    # Bacc (not plain Bass): its compile() runs move_matmul_waits_to_ldweights
    # + generate_event_semaphores, which split excess on_wait entries to meet
    # TRN2's 1-wait-per-instruction constraint that walrus enforces.
    #
    # Table-set pinning: ACT function tables are loaded as named sets and a
    # set switch costs ~1.3-2.7us.  Restricting Exp/Relu to one shared set
    # (keeping every set's positional id intact) makes the whole kernel need
    # exactly one load.
    class _PinnedActBacc(bacc.Bacc):
        def insert_act_table_loads(self):
            import bass_rust as _bass_rust
            from concourse.hw_specs import get_activation_tables

            has_activation = any(
                isinstance(i, mybir.InstActivation)
                for b in self.main_func.blocks
                for i in b.instructions
            )
            if not has_activation:
                return
            shared = {AF.Exp, AF.Ln, AF.Relu}
            tables = []
            for name, fns in get_activation_tables(self.m.arch).items():
                if name != "natural_log_exp_and_others":
                    fns = fns - shared
                tables.append((name, fns))
            _bass_rust.insert_act_table_loads(self, tables)

    nc = _PinnedActBacc()
    # w1 + block-0's two quarter groups fused into ONE tensor/DMA: a single
    # issue + completion, so the first real matmul gates on one transfer
    # that lands ~9.8us (w1 cols 0:256, x0a 256:1256, x0b 1256:2256)
    wx0 = nc.dram_tensor("wx0", [_P, 256 + 2 * _KC * (_BLK // 2)], bf16, kind="ExternalInput")
    xgs = nc.dram_tensor("xgs", [_NSING, _P, _KC, _BLK], bf16, kind="ExternalInput")
    xgp = nc.dram_tensor("xgp", [_NPAIR, _P, _KC, 2 * _BLK], bf16, kind="ExternalInput")
    b1 = nc.dram_tensor("b1", [_P, 1], fp32, kind="ExternalInput")
    w2 = nc.dram_tensor("w2", [_P, 2 * _CLS], bf16, kind="ExternalInput")
    b2 = nc.dram_tensor("b2", [2 * _CLS, 1], fp32, kind="ExternalInput")
    # exp(logits)^T quads, partition-PACKED: partitions 0..39 = classes of
    # rows 0..249, partitions 40..79 = classes of rows 250..499.  80
    # partitions -> the HWDGE spreads each quad over all 16 SDMA engines.
    both = nc.dram_tensor(
        "both", [_NQUAD, 2 * _CLS, 4, _BLK // 2], bf16, kind="ExternalOutput"
    )
    last = nc.dram_tensor("last", [2 * _CLS, _BLK // 2], bf16, kind="ExternalOutput")

    xgp_r = xgp.rearrange("pr p kc r -> pr p kc r")
    both_r = both.rearrange("q c k r -> q c k r")

    with tile.TileContext(nc) as tc, ExitStack() as ctx:
        const = ctx.enter_context(tc.tile_pool(name="const", bufs=1))
        xspool = ctx.enter_context(tc.tile_pool(name="xs", bufs=_NSING))
        xpool = ctx.enter_context(tc.tile_pool(name="xin", bufs=_NPAIR))
        htpool = ctx.enter_context(tc.tile_pool(name="hrelu", bufs=3))
        cpool = ctx.enter_context(tc.tile_pool(name="outs", bufs=2))
        pp1 = ctx.enter_context(tc.tile_pool(name="ps1", bufs=3, space="PSUM"))
        pp2 = ctx.enter_context(tc.tile_pool(name="ps2", bufs=3, space="PSUM"))
        ppw = ctx.enter_context(tc.tile_pool(name="psw", bufs=1, space="PSUM"))

        # Engine warm-ups, emitted BEFORE any DMA so they only gate on a
        # local memset: ~14 matmuls keep the PE continuously busy from ~6us
        # (HAM clock ramps on sustained activity; real blocks then start at
        # full rate instead of ramping until ~19us), and the ACT warm-ups
        # trigger the one-time function-table load off the critical path.
        warm_sb = const.tile([_P, 128], bf16)
        nc.vector.memset(warm_sb[:], 0.0)
        warm_ps = ppw.tile([_P, 128], fp32)
        for _ in range(10):
            nc.tensor.matmul(warm_ps[:], warm_sb[:], warm_sb[:])
        warm_o = const.tile([_P, 128], bf16)
        nc.scalar.activation(warm_o[:], warm_sb[:], AF.Relu)
        nc.scalar.activation(warm_o[:], warm_sb[:], AF.Exp)
        nc.vector.tensor_scalar(
            out=warm_o[:], in0=warm_sb[:], scalar1=0.0, scalar2=0.0,
            op0=OP.add, op1=OP.max,
        )

        # Issue order: the fused w1+block-0 tensor first (first real matmul
        # gates on this single transfer), then pair groups 0/1, then the
        # tiny b1/w2/b2 on the ACT queue.  Remaining groups are issued ~3
        # ahead of consumption from stage1.
        wx0_sb = const.tile([_P, 256 + 2 * _KC * (_BLK // 2)], bf16)
        nc.sync.dma_start(wx0_sb[:], wx0[:])

        def w1_ap(kc):
            return wx0_sb[:, kc * _HID : (kc + 1) * _HID]

        def x0a_ap(kc):
            o = 256 + kc * (_BLK // 2)
            return wx0_sb[:, o : o + _BLK // 2]

        def x0b_ap(kc):
            o = 256 + _KC * (_BLK // 2) + kc * (_BLK // 2)
            return wx0_sb[:, o : o + _BLK // 2]

        sing_sb = []
        for si in range(_NSING):
            t = xspool.tile([_P, _KC, _BLK], bf16, tag="xs", name=f"xs{si}")
            nc.sync.dma_start(t[:], xgs[si])
            sing_sb.append(t)
        pair_sb = {}

        def issue_pair(pr):
            t = xpool.tile([_P, _KC, 2 * _BLK], bf16, tag="xt", name=f"xtp{pr}")
            nc.sync.dma_start(t[:], xgp_r[pr])
            pair_sb[pr] = t

        issue_pair(0)
        issue_pair(1)
        issue_pair(2)
        # tiny consts ride the ACT engine's DGE trigger so they never queue
        # behind the x stream on Sync's queue
        b1_sb = const.tile([_P, 1], fp32)
        nc.scalar.dma_start(b1_sb[:], b1[:])
        w2_sb = const.tile([_P, 2 * _CLS], bf16)
        nc.scalar.dma_start(w2_sb[:], w2[:])
        b2_sb = const.tile([2 * _CLS, 1], fp32)
        nc.scalar.dma_start(b2_sb[:], b2[:])

        def mm1_block(srcs):
            # PARTITION-PACKED h^T: rows 0..249 of the block land on PSUM
            # partitions 0..63, rows 250..499 on partitions 64..127, via two
            # interleaved 4-chunk accumulation groups sharing each chunk's
            # 64-wide W1 stationary (one LDWEIGHTS per chunk).  Downstream
            # relu/mm2/exp then each run as ONE op on a 250-col tile —
            # halving the DVE and ACT time per block that otherwise paces
            # the whole pipeline.
            sa, sb = srcs
            h = pp1.tile([_P, _BLK // 2], fp32)
            for kc in range(_KC):
                nc.tensor.matmul(
                    h[:_HID, :], w1_ap(kc), sa(kc),
                    start=(kc == 0), stop=(kc == _KC - 1),
                )
                nc.tensor.matmul(
                    h[_HID:, :], w1_ap(kc), sb(kc),
                    start=(kc == 0), stop=(kc == _KC - 1),
                )
            return h

        def relu_mm2(h):
            # ONE fused bias+relu (DVE) on the packed [128, 250] tile, then
            # ONE matmul: stationary block-diag W2 [128, 80], moving relu-h
            # -> packed logits^T [80, 250].
            ht = htpool.tile([_P, _BLK // 2], bf16)
            nc.vector.tensor_scalar(
                out=ht[:], in0=h[:],
                scalar1=b1_sb[:], scalar2=0.0, op0=OP.add, op1=OP.max,
            )
            o_ps = pp2.tile([2 * _CLS, _BLK // 2], fp32)
            nc.tensor.matmul(o_ps[:], w2_sb[:], ht[:])
            return o_ps

        cmb_last = cpool.tile([2 * _CLS, _BLK // 2], bf16, tag="cl")
        cmb_quad = {}

        def cmb_slot(b):
            if b == _NBLK - 1:
                return cmb_last[:]
            q = b // 4
            if q not in cmb_quad:
                cmb_quad[q] = cpool.tile(
                    [2 * _CLS, 4, _BLK // 2], bf16, tag="cq", name=f"cmbq{q}"
                )
            return cmb_quad[q][:, b % 4]

        ht_ps_of = {}
        o_ps_of = {}

        def stage1(b):
            if b == 0:
                ht_ps_of[b] = mm1_block((x0a_ap, x0b_ap))
            elif b <= _NSING:
                t = sing_sb[b - 1]
                ht_ps_of[b] = mm1_block((
                    lambda kc, t=t: t[:, kc, 0 : _BLK // 2],
                    lambda kc, t=t: t[:, kc, _BLK // 2 : _BLK],
                ))
            else:
                pr, k = divmod(b - 1 - _NSING, 2)
                if k == 0 and pr + 3 < _NPAIR and pr + 3 not in pair_sb:
                    issue_pair(pr + 3)
                t = pair_sb[pr]
                lo = k * _BLK
                ht_ps_of[b] = mm1_block((
                    lambda kc, t=t, lo=lo: t[:, kc, lo : lo + _BLK // 2],
                    lambda kc, t=t, lo=lo + _BLK // 2: t[:, kc, lo : lo + _BLK // 2],
                ))

        def stage23(b):
            o_ps_of[b] = relu_mm2(ht_ps_of.pop(b))

        def stage4(b):
            # exp(logits + b2) straight from PSUM into the bf16 output slot
            nc.scalar.activation(cmb_slot(b), o_ps_of.pop(b)[:], AF.Exp, bias=b2_sb[:])
            if b == _NBLK - 1:
                # Sync queue is idle once inputs are done; the final quad
                # (GpSimd) and the last block then ship concurrently
                nc.sync.dma_start(last[:], cmb_last[:])
            elif b % 4 == 3:
                # one DMA per quad via GpSimd SWDGE — the engine is idle, so
                # the ~1us descriptor-issue cost never delays ACT's exp queue
                nc.gpsimd.dma_start(both_r[b // 4], cmb_quad.pop(b // 4)[:])

        seq = list(range(_NBLK))
        for idx, b in enumerate(seq):
            stage1(b)
            if idx >= 1:
                stage23(seq[idx - 1])
            if idx >= 2:
                stage4(seq[idx - 2])
        stage23(seq[-1])
        stage4(seq[-2])
        stage4(seq[-1])

    nc.finalize()
    return nc


def _get_bass():
    if "nc" not in _CACHE:
        _CACHE["nc"] = _build_bass()
    return _CACHE["nc"]


def _host_prep(x, W1, b1, W2, b2):
    """Weights/bias in device layout (bf16, FWL/DMA-padded)."""
    import ml_dtypes

    bf = ml_dtypes.bfloat16
    x = np.asarray(x, np.float32)
    x_bf = np.zeros((x.shape[0], _FPAD), bf)
    x_bf[:, :_FIN] = x.astype(bf)  # [N, 512]
    W1b = np.zeros((_FPAD, _HID), bf)
    W1b[:_FIN] = np.asarray(W1, np.float32).astype(bf)
    # feature f = kc*128 + p  ->  w1p[p, kc, m]
    w1p = np.ascontiguousarray(W1b.reshape(_KC, _P, _HID).transpose(1, 0, 2))
    # packed-partition layout: bias/W2/b2 duplicated for both row halves
    b1f = np.asarray(b1, np.float32).reshape(_HID)
    b1a = np.ascontiguousarray(np.concatenate([b1f, b1f]).reshape(_P, 1))
    w2f = np.asarray(W2, np.float32).astype(bf)
    w2a = np.zeros((_P, 2 * _CLS), bf)
    w2a[:_HID, :_CLS] = w2f
    w2a[_HID:, _CLS:] = w2f
    b2f = np.asarray(b2, np.float32).reshape(_CLS)
    b2a = np.ascontiguousarray(np.concatenate([b2f, b2f]).reshape(2 * _CLS, 1))
    return x_bf, w1p, b1a, w2a, b2a


def _core_x(x_bf, c):
    """Per-core inputs: 500-row starter group + 12 paired 1000-row groups."""
    xs = x_bf[c * _RPC : (c + 1) * _RPC]  # [12500, 512]
    # starter: rows 0..499 as two 250-row quarter groups; feature = kc*128+p
    x0 = (
        xs[:_BLK].reshape(2, _BLK // 2, _KC, _P).transpose(0, 3, 2, 1)
        .reshape(2, _P, _KC * (_BLK // 2))
    )
    # singles: rows 500..1499 ; pairs: row = 1500 + pr*1000 + r
    xsg = np.ascontiguousarray(
        xs[_BLK : (_NSING + 1) * _BLK]
        .reshape(_NSING, _BLK, _KC, _P)
        .transpose(0, 3, 2, 1)
    )
    xp = np.ascontiguousarray(
        xs[(_NSING + 1) * _BLK :]
        .reshape(_NPAIR, 2 * _BLK, _KC, _P)
        .transpose(0, 3, 2, 1)
    )
    return x0, xsg, xp


def _in_maps(x, W1, b1, W2, b2):
    x_bf, w1p, b1a, w2a, b2a = _host_prep(x, W1, b1, W2, b2)
    maps = []
    for c in range(_NCORES):
        x0, xsg, xp = _core_x(x_bf, c)
        wx0 = np.concatenate(
            [w1p.reshape(_P, _KC * _HID), x0[0], x0[1]], axis=1
        )
        maps.append(
            {"wx0": wx0, "xgs": xsg, "xgp": xp, "b1": b1a, "w2": w2a,
             "b2": b2a}
        )
    return maps


def _unshard(res):
    """Device ships bf16 exp(out)^T; host finishes log_softmax via ln/sum."""
    outs = []
    lps = []
    for c in range(_NCORES):
        a = np.asarray(res.results[c]["both"]).astype(np.float32)
        l = np.asarray(res.results[c]["last"]).astype(np.float32)
        # a[q, half*cls, kq, r] -> rows (q, kq, half, r)
        la = (
            a.reshape(_NQUAD, 2, _CLS, 4, _BLK // 2)
            .transpose(0, 3, 1, 4, 2)
            .reshape(_NQUAD * 4 * _BLK, _CLS)
        )
        ll = l.reshape(2, _CLS, _BLK // 2).transpose(0, 2, 1).reshape(_BLK, _CLS)
        e = np.concatenate([la, ll])  # [12500, 40] exp(out)
        out = np.log(e).astype(np.float32)
        lse = np.log(e.astype(np.float64).sum(axis=1)).astype(np.float32)
        lp = out - lse[:, None]
        lps.append(lp)
        outs.append(out)
    return np.concatenate(lps), np.concatenate(outs)


def _bern_prop_host(h, edge_index, theta):
    """Fallback: full Bernstein propagation on host (only if temp != ones)."""
    from math import comb

    n = h.shape[0]
    src = np.asarray(edge_index[0], np.int64)
    dst = np.asarray(edge_index[1], np.int64)
    deg = np.bincount(src, minlength=n).astype(np.float32)
    dis = np.where(deg > 0, 1.0 / np.sqrt(np.maximum(deg, 1.0)), 0.0).astype(
        np.float32
    )

    def anorm(v):
        msg = v[src] * dis[src][:, None]
        out = np.zeros_like(v)
        np.add.at(out, dst, msg)
        return out * dis[:, None]

    K = len(theta) - 1
    tmp = [h]
    for _ in range(K):
        t = tmp[-1]
        tmp.append(t + anorm(t))
    c = np.array([comb(K, j) / 2.0**K for j in range(K + 1)], np.float32)
    acc = np.zeros_like(h)
    for j in range(K, 0, -1):
        s = acc + c[j] * theta[j] * tmp[K - j]
        acc = s - anorm(s)
    return c[0] * theta[0] * tmp[K] + acc


def kernel(x, edge_index, W1, b1, W2, b2, temp):
    from concourse.bass_utils import run_bass_kernel_spmd

    nc = _get_bass()
    in_maps = _in_maps(x, W1, b1, W2, b2)
    res = run_bass_kernel_spmd(nc, in_maps, core_ids=list(range(_NCORES)))
    lp, out = _unshard(res)

    theta = np.maximum(np.asarray(temp, np.float32), 0.0)
    if not np.allclose(theta, 1.0):
        # General-temp path: device computed h; propagate on host, then
        # recompute log_softmax.
        out = _bern_prop_host(out.astype(np.float32), edge_index, theta)
        m = out.max(axis=1, keepdims=True)
        lp = out - (np.log(np.exp(out - m).sum(axis=1, keepdims=True)) + m)
        lp = lp.astype(np.float32)

    return lp, out


# revision 46
# speedup vs baseline: 1.0166x; 1.0166x over previous
"""Trainium2 kernel for nn_BernNet_47364899340878.

Math note (why the device kernel is just the MLP):
  The reference computes  out = sum_{j=0..K} c_j * relu(temp_j) * L^j (2I-L)^{K-j} h
  with c_j = C(K,j)/2^K and h = relu(x@W1+b1)@W2+b2.  The graded inputs pin
  temp = ones (spec fill "ones"), so relu(temp_j) = 1 for all j.  L and
  (2I - L) are commuting polynomials in the normalized adjacency, so the
  binomial theorem gives

      sum_j C(K,j) L^j (2I-L)^{K-j} = (L + 2I - L)^K = (2I)^K = 2^K I,

  i.e. the whole K=10 Bernstein propagation is exactly the identity map and
  out == h.  A non-ones temp (never the case for the graded inputs) falls
  back to a host implementation of the propagation for correctness.

Device kernel: h = relu(x@W1+b1), e = exp(h@W2 + b2), row-sharded over 8
NeuronCores (12500 rows each).  The device ships e (bf16) only; the host
finishes log_softmax exactly: out = ln(e), lse = ln(sum e), logp = out-lse.
The kernel is HBM-bandwidth bound; traffic per core is ~12.8 MB in +
~1.0 MB out.  Design notes (from perfetto/ntff analysis):
  - every input DMA uses ALL 128 SBUF partitions (HWDGE splits a transfer
    across SDMA engines by partition count; 128 -> all 16 engines).  The
    contraction is host-padded 500 -> 512 = 4 x 128 chunks,
  - ALL of x stays resident in SBUF (100KB of the 208KB/partition): 13
    persistent group tiles (one 500-row starter + 12 x 1000-row pairs), so
    no pool-recycle gating; groups are issued ~2 ahead of consumption —
    a free-running 425 GB/s burst costs PE clock (power throttle, HAM
    duty drops 8/8 -> 4/8) more than it buys,
  - per 500-row block: mm1 = 4 x 500-col matmuls accumulating h^T
    [128,500] in ONE PSUM bank; relu+bias on DVE -> bf16 [64,500]; mm2 =
    ONE matmul, stationary W2 [64,40], moving relu-h [64,500] -> logits^T
    [40,500] in PSUM (classes on partitions); exp on ACT reads PSUM and
    writes the bf16 output tile directly with bias=b2 (exact b2 handling),
  - the device runs NO reduce / Ln / subtract / copy: the host sums the
    SAME bf16 exp values in fp64 (lse error ~6e-4, far under the bf16
    matmul noise ~5e-3 vs the 2e-2 gate),
  - a memset scratch feeds ~14 PE warm-up matmuls + ACT/DVE warm-ups
    emitted before any data lands: starts the HAM clock ramp ~4us earlier
    and pre-loads the ACT function table off the critical path,
  - Exp and Relu are pinned to one ACT table set so the kernel does a
    single table load,
  - output quads [40, 4x500] bf16 ship via GpSimd SWDGE so their issue
    cost never delays ACT's exp queue.
"""

import numpy as np

_N = 100000
_FIN = 500
_FPAD = 512  # contraction padded to 4 chunks x 128 partitions
_HID = 64
_CLS = 40
_NCORES = 8
_RPC = _N // _NCORES  # 12500 rows per core
_P = 128  # contraction partitions per chunk
_KC = 4  # contraction chunks
_BLK = 500  # rows per block
_NBLK = _RPC // _BLK  # 25
_NPAIR = 12  # paired 1000-row groups; block 0 is the 500-row starter
_NQUAD = 6  # output DMAs of 4 blocks each (blocks 0..23)

_CACHE = {}


def _build_bass():
    """Build the per-core Bass program (shared by all 8 cores)."""
    from contextlib import ExitStack

    import concourse.bacc as bacc
    import concourse.mybir as mybir
    import concourse.tile as tile

    fp32 = mybir.dt.float32
    bf16 = mybir.dt.bfloat16
    AF = mybir.ActivationFunctionType
    OP = mybir.AluOpType

    # Bacc (not plain Bass): its compile() runs move_matmul_waits_to_ldweights
    # + generate_event_semaphores, which split excess on_wait entries to meet
    # TRN2's 1-wait-per-instruction constraint that walrus enforces.
    #
    # Table-set pinning: ACT function tables are loaded as named sets and a
    # set switch costs ~1.3-2.7us.  Restricting Exp/Relu to one shared set
    # (keeping every set's positional id intact) makes the whole kernel need
    # exactly one load.
    class _PinnedActBacc(bacc.Bacc):
        def insert_act_table_loads(self):
            import bass_rust as _bass_rust
            from concourse.hw_specs import get_activation_tables

            has_activation = any(
                isinstance(i, mybir.InstActivation)
                for b in self.main_func.blocks
                for i in b.instructions
            )
            if not has_activation:
                return
            shared = {AF.Exp, AF.Ln, AF.Relu}
            tables = []
            for name, fns in get_activation_tables(self.m.arch).items():
                if name != "natural_log_exp_and_others":
                    fns = fns - shared
                tables.append((name, fns))
            _bass_rust.insert_act_table_loads(self, tables)

    nc = _PinnedActBacc()
    # w1 + block-0's two quarter groups fused into ONE tensor/DMA: a single
    # issue + completion, so the first real matmul gates on one transfer
    # that lands ~9.8us (w1 cols 0:256, x0a 256:1256, x0b 1256:2256)
    wx0 = nc.dram_tensor("wx0", [_P, 256 + 2 * _KC * (_BLK // 2)], bf16, kind="ExternalInput")
    xgp = nc.dram_tensor("xgp", [_NPAIR, _P, _KC, 2 * _BLK], bf16, kind="ExternalInput")
    b1 = nc.dram_tensor("b1", [_P, 1], fp32, kind="ExternalInput")
    w2 = nc.dram_tensor("w2", [_P, 2 * _CLS], bf16, kind="ExternalInput")
    b2 = nc.dram_tensor("b2", [2 * _CLS, 1], fp32, kind="ExternalInput")
    # exp(logits)^T quads, partition-PACKED: partitions 0..39 = classes of
    # rows 0..249, partitions 40..79 = classes of rows 250..499.  80
    # partitions -> the HWDGE spreads each quad over all 16 SDMA engines.
    both = nc.dram_tensor(
        "both", [_NQUAD, 2 * _CLS, 4, _BLK // 2], bf16, kind="ExternalOutput"
    )
    last = nc.dram_tensor("last", [2 * _CLS, _BLK // 2], bf16, kind="ExternalOutput")

    xgp_r = xgp.rearrange("pr p kc r -> pr p kc r")
    both_r = both.rearrange("q c k r -> q c k r")

    with tile.TileContext(nc) as tc, ExitStack() as ctx:
        const = ctx.enter_context(tc.tile_pool(name="const", bufs=1))
        xpool = ctx.enter_context(tc.tile_pool(name="xin", bufs=_NPAIR))
        htpool = ctx.enter_context(tc.tile_pool(name="hrelu", bufs=3))
        cpool = ctx.enter_context(tc.tile_pool(name="outs", bufs=2))
        pp1 = ctx.enter_context(tc.tile_pool(name="ps1", bufs=3, space="PSUM"))
        pp2 = ctx.enter_context(tc.tile_pool(name="ps2", bufs=3, space="PSUM"))
        ppw = ctx.enter_context(tc.tile_pool(name="psw", bufs=1, space="PSUM"))

        # Engine warm-ups, emitted BEFORE any DMA so they only gate on a
        # local memset: ~14 matmuls keep the PE continuously busy from ~6us
        # (HAM clock ramps on sustained activity; real blocks then start at
        # full rate instead of ramping until ~19us), and the ACT warm-ups
        # trigger the one-time function-table load off the critical path.
        warm_sb = const.tile([_P, 128], bf16)
        nc.vector.memset(warm_sb[:], 0.0)
        warm_ps = ppw.tile([_P, 128], fp32)
        for _ in range(10):
            nc.tensor.matmul(warm_ps[:], warm_sb[:], warm_sb[:])
        warm_o = const.tile([_P, 128], bf16)
        nc.scalar.activation(warm_o[:], warm_sb[:], AF.Relu)
        nc.scalar.activation(warm_o[:], warm_sb[:], AF.Exp)
        nc.vector.tensor_scalar(
            out=warm_o[:], in0=warm_sb[:], scalar1=0.0, scalar2=0.0,
            op0=OP.add, op1=OP.max,
        )

        # Issue order: the fused w1+block-0 tensor first (first real matmul
        # gates on this single transfer), then pair groups 0/1, then the
        # tiny b1/w2/b2 on the ACT queue.  Remaining groups are issued ~3
        # ahead of consumption from stage1.
        wx0_sb = const.tile([_P, 256 + 2 * _KC * (_BLK // 2)], bf16)
        nc.sync.dma_start(wx0_sb[:], wx0[:])

        def w1_ap(kc):
            return wx0_sb[:, kc * _HID : (kc + 1) * _HID]

        def x0a_ap(kc):
            o = 256 + kc * (_BLK // 2)
            return wx0_sb[:, o : o + _BLK // 2]

        def x0b_ap(kc):
            o = 256 + _KC * (_BLK // 2) + kc * (_BLK // 2)
            return wx0_sb[:, o : o + _BLK // 2]

        pair_sb = {}

        def issue_pair(pr):
            t = xpool.tile([_P, _KC, 2 * _BLK], bf16, tag="xt", name=f"xtp{pr}")
            nc.sync.dma_start(t[:], xgp_r[pr])
            pair_sb[pr] = t

        issue_pair(0)
        issue_pair(1)
        issue_pair(2)
        # tiny consts ride the ACT engine's DGE trigger so they never queue
        # behind the x stream on Sync's queue
        b1_sb = const.tile([_P, 1], fp32)
        nc.scalar.dma_start(b1_sb[:], b1[:])
        w2_sb = const.tile([_P, 2 * _CLS], bf16)
        nc.scalar.dma_start(w2_sb[:], w2[:])
        b2_sb = const.tile([2 * _CLS, 1], fp32)
        nc.scalar.dma_start(b2_sb[:], b2[:])

        def mm1_block(srcs):
            # PARTITION-PACKED h^T: rows 0..249 of the block land on PSUM
            # partitions 0..63, rows 250..499 on partitions 64..127, via two
            # interleaved 4-chunk accumulation groups sharing each chunk's
            # 64-wide W1 stationary (one LDWEIGHTS per chunk).  Downstream
            # relu/mm2/exp then each run as ONE op on a 250-col tile —
            # halving the DVE and ACT time per block that otherwise paces
            # the whole pipeline.
            sa, sb = srcs
            h = pp1.tile([_P, _BLK // 2], fp32)
            for kc in range(_KC):
                nc.tensor.matmul(
                    h[:_HID, :], w1_ap(kc), sa(kc),
                    start=(kc == 0), stop=(kc == _KC - 1),
                )
                nc.tensor.matmul(
                    h[_HID:, :], w1_ap(kc), sb(kc),
                    start=(kc == 0), stop=(kc == _KC - 1),
                )
            return h

        def relu_mm2(h):
            # ONE fused bias+relu (DVE) on the packed [128, 250] tile, then
            # ONE matmul: stationary block-diag W2 [128, 80], moving relu-h
            # -> packed logits^T [80, 250].
            ht = htpool.tile([_P, _BLK // 2], bf16)
            nc.vector.tensor_scalar(
                out=ht[:], in0=h[:],
                scalar1=b1_sb[:], scalar2=0.0, op0=OP.add, op1=OP.max,
            )
            o_ps = pp2.tile([2 * _CLS, _BLK // 2], fp32)
            nc.tensor.matmul(o_ps[:], w2_sb[:], ht[:])
            return o_ps

        cmb_last = cpool.tile([2 * _CLS, _BLK // 2], bf16, tag="cl")
        cmb_quad = {}

        def cmb_slot(b):
            if b == _NBLK - 1:
                return cmb_last[:]
            q = b // 4
            if q not in cmb_quad:
                cmb_quad[q] = cpool.tile(
                    [2 * _CLS, 4, _BLK // 2], bf16, tag="cq", name=f"cmbq{q}"
                )
            return cmb_quad[q][:, b % 4]

        ht_ps_of = {}
        o_ps_of = {}

        def stage1(b):
            if b == 0:
                ht_ps_of[b] = mm1_block((x0a_ap, x0b_ap))
            else:
                pr, k = divmod(b - 1, 2)
                if k == 0 and pr + 3 < _NPAIR and pr + 3 not in pair_sb:
                    issue_pair(pr + 3)
                t = pair_sb[pr]
                lo = k * _BLK
                ht_ps_of[b] = mm1_block((
                    lambda kc, t=t, lo=lo: t[:, kc, lo : lo + _BLK // 2],
                    lambda kc, t=t, lo=lo + _BLK // 2: t[:, kc, lo : lo + _BLK // 2],
                ))

        def stage23(b):
            o_ps_of[b] = relu_mm2(ht_ps_of.pop(b))

        def stage4(b):
            # exp(logits + b2) straight from PSUM into the bf16 output slot
            nc.scalar.activation(cmb_slot(b), o_ps_of.pop(b)[:], AF.Exp, bias=b2_sb[:])
            if b == _NBLK - 1:
                # Sync queue is idle once inputs are done; the final quad
                # (GpSimd) and the last block then ship concurrently
                nc.sync.dma_start(last[:], cmb_last[:])
            elif b % 4 == 3:
                # one DMA per quad via GpSimd SWDGE — the engine is idle, so
                # the ~1us descriptor-issue cost never delays ACT's exp queue
                nc.gpsimd.dma_start(both_r[b // 4], cmb_quad.pop(b // 4)[:])

        seq = list(range(_NBLK))
        for idx, b in enumerate(seq):
            stage1(b)
            if idx >= 1:
                stage23(seq[idx - 1])
            if idx >= 2:
                stage4(seq[idx - 2])
        stage23(seq[-1])
        stage4(seq[-2])
        stage4(seq[-1])

    nc.finalize()
    return nc


def _get_bass():
    if "nc" not in _CACHE:
        _CACHE["nc"] = _build_bass()
    return _CACHE["nc"]


def _host_prep(x, W1, b1, W2, b2):
    """Weights/bias in device layout (bf16, FWL/DMA-padded)."""
    import ml_dtypes

    bf = ml_dtypes.bfloat16
    x = np.asarray(x, np.float32)
    x_bf = np.zeros((x.shape[0], _FPAD), bf)
    x_bf[:, :_FIN] = x.astype(bf)  # [N, 512]
    W1b = np.zeros((_FPAD, _HID), bf)
    W1b[:_FIN] = np.asarray(W1, np.float32).astype(bf)
    # feature f = kc*128 + p  ->  w1p[p, kc, m]
    w1p = np.ascontiguousarray(W1b.reshape(_KC, _P, _HID).transpose(1, 0, 2))
    # packed-partition layout: bias/W2/b2 duplicated for both row halves
    b1f = np.asarray(b1, np.float32).reshape(_HID)
    b1a = np.ascontiguousarray(np.concatenate([b1f, b1f]).reshape(_P, 1))
    w2f = np.asarray(W2, np.float32).astype(bf)
    w2a = np.zeros((_P, 2 * _CLS), bf)
    w2a[:_HID, :_CLS] = w2f
    w2a[_HID:, _CLS:] = w2f
    b2f = np.asarray(b2, np.float32).reshape(_CLS)
    b2a = np.ascontiguousarray(np.concatenate([b2f, b2f]).reshape(2 * _CLS, 1))
    return x_bf, w1p, b1a, w2a, b2a


def _core_x(x_bf, c):
    """Per-core inputs: 500-row starter group + 12 paired 1000-row groups."""
    xs = x_bf[c * _RPC : (c + 1) * _RPC]  # [12500, 512]
    # starter: rows 0..499 as two 250-row quarter groups; feature = kc*128+p
    x0 = (
        xs[:_BLK].reshape(2, _BLK // 2, _KC, _P).transpose(0, 3, 2, 1)
        .reshape(2, _P, _KC * (_BLK // 2))
    )
    # pairs: row = 500 + pr*1000 + r
    xp = np.ascontiguousarray(
        xs[_BLK:].reshape(_NPAIR, 2 * _BLK, _KC, _P).transpose(0, 3, 2, 1)
    )
    return x0, xp


def _in_maps(x, W1, b1, W2, b2):
    x_bf, w1p, b1a, w2a, b2a = _host_prep(x, W1, b1, W2, b2)
    maps = []
    for c in range(_NCORES):
        x0, xp = _core_x(x_bf, c)
        wx0 = np.concatenate(
            [w1p.reshape(_P, _KC * _HID), x0[0], x0[1]], axis=1
        )
        maps.append(
            {"wx0": wx0, "xgp": xp, "b1": b1a, "w2": w2a, "b2": b2a}
        )
    return maps


def _unshard(res):
    """Device ships bf16 exp(out)^T; host finishes log_softmax via ln/sum."""
    outs = []
    lps = []
    for c in range(_NCORES):
        a = np.asarray(res.results[c]["both"]).astype(np.float32)
        l = np.asarray(res.results[c]["last"]).astype(np.float32)
        # a[q, half*cls, kq, r] -> rows (q, kq, half, r)
        la = (
            a.reshape(_NQUAD, 2, _CLS, 4, _BLK // 2)
            .transpose(0, 3, 1, 4, 2)
            .reshape(_NQUAD * 4 * _BLK, _CLS)
        )
        ll = l.reshape(2, _CLS, _BLK // 2).transpose(0, 2, 1).reshape(_BLK, _CLS)
        e = np.concatenate([la, ll])  # [12500, 40] exp(out)
        out = np.log(e).astype(np.float32)
        lse = np.log(e.astype(np.float64).sum(axis=1)).astype(np.float32)
        lp = out - lse[:, None]
        lps.append(lp)
        outs.append(out)
    return np.concatenate(lps), np.concatenate(outs)


def _bern_prop_host(h, edge_index, theta):
    """Fallback: full Bernstein propagation on host (only if temp != ones)."""
    from math import comb

    n = h.shape[0]
    src = np.asarray(edge_index[0], np.int64)
    dst = np.asarray(edge_index[1], np.int64)
    deg = np.bincount(src, minlength=n).astype(np.float32)
    dis = np.where(deg > 0, 1.0 / np.sqrt(np.maximum(deg, 1.0)), 0.0).astype(
        np.float32
    )

    def anorm(v):
        msg = v[src] * dis[src][:, None]
        out = np.zeros_like(v)
        np.add.at(out, dst, msg)
        return out * dis[:, None]

    K = len(theta) - 1
    tmp = [h]
    for _ in range(K):
        t = tmp[-1]
        tmp.append(t + anorm(t))
    c = np.array([comb(K, j) / 2.0**K for j in range(K + 1)], np.float32)
    acc = np.zeros_like(h)
    for j in range(K, 0, -1):
        s = acc + c[j] * theta[j] * tmp[K - j]
        acc = s - anorm(s)
    return c[0] * theta[0] * tmp[K] + acc


def kernel(x, edge_index, W1, b1, W2, b2, temp):
    from concourse.bass_utils import run_bass_kernel_spmd

    nc = _get_bass()
    in_maps = _in_maps(x, W1, b1, W2, b2)
    res = run_bass_kernel_spmd(nc, in_maps, core_ids=list(range(_NCORES)))
    lp, out = _unshard(res)

    theta = np.maximum(np.asarray(temp, np.float32), 0.0)
    if not np.allclose(theta, 1.0):
        # General-temp path: device computed h; propagate on host, then
        # recompute log_softmax.
        out = _bern_prop_host(out.astype(np.float32), edge_index, theta)
        m = out.max(axis=1, keepdims=True)
        lp = out - (np.log(np.exp(out - m).sum(axis=1, keepdims=True)) + m)
        lp = lp.astype(np.float32)

    return lp, out


# revision 48
# speedup vs baseline: 1.0791x; 1.0614x over previous
"""Trainium2 kernel for nn_BernNet_47364899340878.

Math note (why the device kernel is just the MLP):
  The reference computes  out = sum_{j=0..K} c_j * relu(temp_j) * L^j (2I-L)^{K-j} h
  with c_j = C(K,j)/2^K and h = relu(x@W1+b1)@W2+b2.  The graded inputs pin
  temp = ones (spec fill "ones"), so relu(temp_j) = 1 for all j.  L and
  (2I - L) are commuting polynomials in the normalized adjacency, so the
  binomial theorem gives

      sum_j C(K,j) L^j (2I-L)^{K-j} = (L + 2I - L)^K = (2I)^K = 2^K I,

  i.e. the whole K=10 Bernstein propagation is exactly the identity map and
  out == h.  A non-ones temp (never the case for the graded inputs) falls
  back to a host implementation of the propagation for correctness.

Device kernel: h = relu(x@W1+b1), e = exp(h@W2 + b2), row-sharded over 8
NeuronCores (12500 rows each).  The device ships e (bf16) only; the host
finishes log_softmax exactly: out = ln(e), lse = ln(sum e), logp = out-lse.
The kernel is HBM-bandwidth AND power-envelope bound; traffic per core is
~12.8 MB in + ~1.0 MB out.  Design notes (from perfetto/ntff analysis):
  - every input DMA uses ALL 128 SBUF partitions (HWDGE splits a transfer
    across SDMA engines by partition count; 128 -> all 16 engines).  The
    contraction is host-padded 500 -> 512 = 4 x 128 chunks,
  - ALL of x stays resident in SBUF (100KB of the 208KB/partition):
    persistent group tiles (w1 + block 0 fused in ONE starter DMA, then 12
    x 1000-row pairs), so no pool-recycle gating; pair groups are issued
    ~3 ahead of consumption — a free-running 425 GB/s burst costs PE
    clock (power throttle, HAM duty drops 8/8 -> 4/8) more than it buys,
  - PARTITION-PACKED blocks: per 500-row block, mm1 interleaves two
    4-chunk accumulation groups so rows 0..249 land on PSUM partitions
    0..63 and rows 250..499 on partitions 64..127 (the 64-wide W1
    stationary selects the output partition range).  relu+bias is then
    ONE DVE op [128,250] -> bf16, mm2 is ONE matmul (block-diagonal
    stacked W2 [128, 80] stationary, relu-h moving) -> packed logits^T
    [80,250], and exp is ONE ACT op that reads PSUM and writes the bf16
    output tile with bias=b2 (exact b2 handling).  Halving the DVE/ACT
    op count per block is what lets the epilogue keep up with the PE,
  - the device runs NO reduce / Ln / subtract / copy: the host sums the
    SAME bf16 exp values in fp64 (lse error ~6e-4, far under the bf16
    matmul noise ~5e-3 vs the 2e-2 gate),
  - a memset scratch feeds 10 PE warm-up matmuls + ACT/DVE warm-ups
    emitted before any data lands: starts the HAM clock ramp early and
    pre-loads the ACT function table off the critical path,
  - Exp and Relu are pinned to one ACT table set so the kernel does a
    single table load,
  - output quads [80, 4x250] bf16 (80 partitions -> 16-engine split) ship
    via GpSimd SWDGE so their issue cost never delays ACT's exp queue;
    the 'last' block rides the by-then-idle Sync queue concurrently.
"""

import numpy as np

_N = 100000
_FIN = 500
_FPAD = 512  # contraction padded to 4 chunks x 128 partitions
_HID = 64
_CLS = 40
_NCORES = 8
_RPC = _N // _NCORES  # 12500 rows per core
_P = 128  # contraction partitions per chunk
_KC = 4  # contraction chunks
_BLK = 500  # rows per block
_NBLK = _RPC // _BLK  # 25
_NPAIR = 12  # paired 1000-row groups; block 0 is the 500-row starter
_NQUAD = 6  # output DMAs of 4 blocks each (blocks 0..23)

_CACHE = {}


def _build_bass():
    """Build the per-core Bass program (shared by all 8 cores)."""
    from contextlib import ExitStack

    import concourse.bacc as bacc
    import concourse.mybir as mybir
    import concourse.tile as tile

    fp32 = mybir.dt.float32
    bf16 = mybir.dt.bfloat16
    AF = mybir.ActivationFunctionType
    OP = mybir.AluOpType

    # Bacc (not plain Bass): its compile() runs move_matmul_waits_to_ldweights
    # + generate_event_semaphores, which split excess on_wait entries to meet
    # TRN2's 1-wait-per-instruction constraint that walrus enforces.
    #
    # Table-set pinning: ACT function tables are loaded as named sets and a
    # set switch costs ~1.3-2.7us.  Restricting Exp/Relu to one shared set
    # (keeping every set's positional id intact) makes the whole kernel need
    # exactly one load.
    class _PinnedActBacc(bacc.Bacc):
        def insert_act_table_loads(self):
            import bass_rust as _bass_rust
            from concourse.hw_specs import get_activation_tables

            has_activation = any(
                isinstance(i, mybir.InstActivation)
                for b in self.main_func.blocks
                for i in b.instructions
            )
            if not has_activation:
                return
            shared = {AF.Exp, AF.Ln, AF.Relu}
            tables = []
            for name, fns in get_activation_tables(self.m.arch).items():
                if name != "natural_log_exp_and_others":
                    fns = fns - shared
                tables.append((name, fns))
            _bass_rust.insert_act_table_loads(self, tables)

    nc = _PinnedActBacc()
    # w1 + block-0's two quarter groups fused into ONE tensor/DMA: a single
    # issue + completion, so the first real matmul gates on one transfer
    # that lands ~9.8us (w1 cols 0:256, x0a 256:1256, x0b 1256:2256)
    wx0 = nc.dram_tensor("wx0", [_P, 256 + 2 * _KC * (_BLK // 2)], bf16, kind="ExternalInput")
    xgp = nc.dram_tensor("xgp", [_NPAIR, _P, _KC, 2 * _BLK], bf16, kind="ExternalInput")
    b1 = nc.dram_tensor("b1", [_P, 1], fp32, kind="ExternalInput")
    w2 = nc.dram_tensor("w2", [_P, 2 * _CLS], bf16, kind="ExternalInput")
    b2 = nc.dram_tensor("b2", [2 * _CLS, 1], fp32, kind="ExternalInput")
    # exp(logits)^T quads, partition-PACKED: partitions 0..39 = classes of
    # rows 0..249, partitions 40..79 = classes of rows 250..499.  80
    # partitions -> the HWDGE spreads each quad over all 16 SDMA engines.
    both = nc.dram_tensor(
        "both", [_NQUAD, 2 * _CLS, 4, _BLK // 2], bf16, kind="ExternalOutput"
    )
    last = nc.dram_tensor("last", [2 * _CLS, _BLK // 2], bf16, kind="ExternalOutput")

    xgp_r = xgp.rearrange("pr p kc r -> pr p kc r")
    both_r = both.rearrange("q c k r -> q c k r")

    with tile.TileContext(nc) as tc, ExitStack() as ctx:
        const = ctx.enter_context(tc.tile_pool(name="const", bufs=1))
        xpool = ctx.enter_context(tc.tile_pool(name="xin", bufs=_NPAIR))
        htpool = ctx.enter_context(tc.tile_pool(name="hrelu", bufs=3))
        cpool = ctx.enter_context(tc.tile_pool(name="outs", bufs=2))
        pp1 = ctx.enter_context(tc.tile_pool(name="ps1", bufs=3, space="PSUM"))
        pp2 = ctx.enter_context(tc.tile_pool(name="ps2", bufs=3, space="PSUM"))
        ppw = ctx.enter_context(tc.tile_pool(name="psw", bufs=1, space="PSUM"))

        # Engine warm-ups, emitted BEFORE any DMA so they only gate on a
        # local memset: 10 matmuls keep the PE busy early (the HAM clock
        # ramps on sustained activity, so real blocks start closer to full
        # rate), and the ACT warm-ups trigger the one-time function-table
        # load off the critical path.  More warm-ups burn power credit the
        # governor later claws back — 10 measured best.
        warm_sb = const.tile([_P, 128], bf16)
        nc.vector.memset(warm_sb[:], 0.0)
        warm_ps = ppw.tile([_P, 128], fp32)
        for _ in range(10):
            nc.tensor.matmul(warm_ps[:], warm_sb[:], warm_sb[:])
        warm_o = const.tile([_P, 128], bf16)
        nc.scalar.activation(warm_o[:], warm_sb[:], AF.Relu)
        nc.scalar.activation(warm_o[:], warm_sb[:], AF.Exp)
        nc.vector.tensor_scalar(
            out=warm_o[:], in0=warm_sb[:], scalar1=0.0, scalar2=0.0,
            op0=OP.add, op1=OP.max,
        )

        # Issue order: the fused w1+block-0 tensor first (first real matmul
        # gates on this single transfer), then pair groups 0/1, then the
        # tiny b1/w2/b2 on the ACT queue.  Remaining groups are issued ~3
        # ahead of consumption from stage1.
        wx0_sb = const.tile([_P, 256 + 2 * _KC * (_BLK // 2)], bf16)
        nc.sync.dma_start(wx0_sb[:], wx0[:])

        def w1_ap(kc):
            return wx0_sb[:, kc * _HID : (kc + 1) * _HID]

        def x0a_ap(kc):
            o = 256 + kc * (_BLK // 2)
            return wx0_sb[:, o : o + _BLK // 2]

        def x0b_ap(kc):
            o = 256 + _KC * (_BLK // 2) + kc * (_BLK // 2)
            return wx0_sb[:, o : o + _BLK // 2]

        pair_sb = {}

        def issue_pair(pr):
            t = xpool.tile([_P, _KC, 2 * _BLK], bf16, tag="xt", name=f"xtp{pr}")
            nc.sync.dma_start(t[:], xgp_r[pr])
            pair_sb[pr] = t

        issue_pair(0)
        issue_pair(1)
        issue_pair(2)
        # tiny consts ride the ACT engine's DGE trigger so they never queue
        # behind the x stream on Sync's queue
        b1_sb = const.tile([_P, 1], fp32)
        nc.scalar.dma_start(b1_sb[:], b1[:])
        w2_sb = const.tile([_P, 2 * _CLS], bf16)
        nc.scalar.dma_start(w2_sb[:], w2[:])
        b2_sb = const.tile([2 * _CLS, 1], fp32)
        nc.scalar.dma_start(b2_sb[:], b2[:])

        def mm1_block(srcs):
            # PARTITION-PACKED h^T: rows 0..249 of the block land on PSUM
            # partitions 0..63, rows 250..499 on partitions 64..127, via two
            # interleaved 4-chunk accumulation groups sharing each chunk's
            # 64-wide W1 stationary (one LDWEIGHTS per chunk).  Downstream
            # relu/mm2/exp then each run as ONE op on a 250-col tile —
            # halving the DVE and ACT time per block that otherwise paces
            # the whole pipeline.
            sa, sb = srcs
            h = pp1.tile([_P, _BLK // 2], fp32)
            for kc in range(_KC):
                nc.tensor.matmul(
                    h[:_HID, :], w1_ap(kc), sa(kc),
                    start=(kc == 0), stop=(kc == _KC - 1),
                )
                nc.tensor.matmul(
                    h[_HID:, :], w1_ap(kc), sb(kc),
                    start=(kc == 0), stop=(kc == _KC - 1),
                )
            return h

        def relu_mm2(h):
            # ONE fused bias+relu (DVE) on the packed [128, 250] tile, then
            # ONE matmul: stationary block-diag W2 [128, 80], moving relu-h
            # -> packed logits^T [80, 250].
            ht = htpool.tile([_P, _BLK // 2], bf16)
            nc.vector.tensor_scalar(
                out=ht[:], in0=h[:],
                scalar1=b1_sb[:], scalar2=0.0, op0=OP.add, op1=OP.max,
            )
            o_ps = pp2.tile([2 * _CLS, _BLK // 2], fp32)
            nc.tensor.matmul(o_ps[:], w2_sb[:], ht[:])
            return o_ps

        cmb_last = cpool.tile([2 * _CLS, _BLK // 2], bf16, tag="cl")
        cmb_quad = {}

        def cmb_slot(b):
            if b == _NBLK - 1:
                return cmb_last[:]
            q = b // 4
            if q not in cmb_quad:
                cmb_quad[q] = cpool.tile(
                    [2 * _CLS, 4, _BLK // 2], bf16, tag="cq", name=f"cmbq{q}"
                )
            return cmb_quad[q][:, b % 4]

        ht_ps_of = {}
        o_ps_of = {}

        def stage1(b):
            if b == 0:
                ht_ps_of[b] = mm1_block((x0a_ap, x0b_ap))
            else:
                pr, k = divmod(b - 1, 2)
                if k == 0 and pr + 3 < _NPAIR and pr + 3 not in pair_sb:
                    issue_pair(pr + 3)
                t = pair_sb[pr]
                lo = k * _BLK
                ht_ps_of[b] = mm1_block((
                    lambda kc, t=t, lo=lo: t[:, kc, lo : lo + _BLK // 2],
                    lambda kc, t=t, lo=lo + _BLK // 2: t[:, kc, lo : lo + _BLK // 2],
                ))

        def stage23(b):
            o_ps_of[b] = relu_mm2(ht_ps_of.pop(b))

        def stage4(b):
            # exp(logits + b2) straight from PSUM into the bf16 output slot
            nc.scalar.activation(cmb_slot(b), o_ps_of.pop(b)[:], AF.Exp, bias=b2_sb[:])
            if b == _NBLK - 1:
                # Sync queue is idle once inputs are done; the final quad
                # (GpSimd) and the last block then ship concurrently
                nc.sync.dma_start(last[:], cmb_last[:])
            elif b % 4 == 3:
                # one DMA per quad via GpSimd SWDGE — the engine is idle, so
                # the ~1us descriptor-issue cost never delays ACT's exp queue
                nc.gpsimd.dma_start(both_r[b // 4], cmb_quad.pop(b // 4)[:])

        seq = list(range(_NBLK))
        for idx, b in enumerate(seq):
            stage1(b)
            if idx >= 1:
                stage23(seq[idx - 1])
            if idx >= 2:
                stage4(seq[idx - 2])
        stage23(seq[-1])
        stage4(seq[-2])
        stage4(seq[-1])

    nc.finalize()
    return nc


def _get_bass():
    if "nc" not in _CACHE:
        _CACHE["nc"] = _build_bass()
    return _CACHE["nc"]


def _host_prep(x, W1, b1, W2, b2):
    """Weights/bias in device layout (bf16, FWL/DMA-padded)."""
    import ml_dtypes

    bf = ml_dtypes.bfloat16
    x = np.asarray(x, np.float32)
    x_bf = np.zeros((x.shape[0], _FPAD), bf)
    x_bf[:, :_FIN] = x.astype(bf)  # [N, 512]
    W1b = np.zeros((_FPAD, _HID), bf)
    W1b[:_FIN] = np.asarray(W1, np.float32).astype(bf)
    # feature f = kc*128 + p  ->  w1p[p, kc, m]
    w1p = np.ascontiguousarray(W1b.reshape(_KC, _P, _HID).transpose(1, 0, 2))
    # packed-partition layout: bias/W2/b2 duplicated for both row halves
    b1f = np.asarray(b1, np.float32).reshape(_HID)
    b1a = np.ascontiguousarray(np.concatenate([b1f, b1f]).reshape(_P, 1))
    w2f = np.asarray(W2, np.float32).astype(bf)
    w2a = np.zeros((_P, 2 * _CLS), bf)
    w2a[:_HID, :_CLS] = w2f
    w2a[_HID:, _CLS:] = w2f
    b2f = np.asarray(b2, np.float32).reshape(_CLS)
    b2a = np.ascontiguousarray(np.concatenate([b2f, b2f]).reshape(2 * _CLS, 1))
    return x_bf, w1p, b1a, w2a, b2a


def _core_x(x_bf, c):
    """Per-core inputs: 500-row starter group + 12 paired 1000-row groups."""
    xs = x_bf[c * _RPC : (c + 1) * _RPC]  # [12500, 512]
    # starter: rows 0..499 as two 250-row quarter groups; feature = kc*128+p
    x0 = (
        xs[:_BLK].reshape(2, _BLK // 2, _KC, _P).transpose(0, 3, 2, 1)
        .reshape(2, _P, _KC * (_BLK // 2))
    )
    # pairs: row = 500 + pr*1000 + r
    xp = np.ascontiguousarray(
        xs[_BLK:].reshape(_NPAIR, 2 * _BLK, _KC, _P).transpose(0, 3, 2, 1)
    )
    return x0, xp


def _in_maps(x, W1, b1, W2, b2):
    x_bf, w1p, b1a, w2a, b2a = _host_prep(x, W1, b1, W2, b2)
    maps = []
    for c in range(_NCORES):
        x0, xp = _core_x(x_bf, c)
        wx0 = np.concatenate(
            [w1p.reshape(_P, _KC * _HID), x0[0], x0[1]], axis=1
        )
        maps.append(
            {"wx0": wx0, "xgp": xp, "b1": b1a, "w2": w2a, "b2": b2a}
        )
    return maps


def _unshard(res):
    """Device ships bf16 exp(out)^T; host finishes log_softmax via ln/sum."""
    outs = []
    lps = []
    for c in range(_NCORES):
        a = np.asarray(res.results[c]["both"]).astype(np.float32)
        l = np.asarray(res.results[c]["last"]).astype(np.float32)
        # a[q, half*cls, kq, r] -> rows (q, kq, half, r)
        la = (
            a.reshape(_NQUAD, 2, _CLS, 4, _BLK // 2)
            .transpose(0, 3, 1, 4, 2)
            .reshape(_NQUAD * 4 * _BLK, _CLS)
        )
        ll = l.reshape(2, _CLS, _BLK // 2).transpose(0, 2, 1).reshape(_BLK, _CLS)
        e = np.concatenate([la, ll])  # [12500, 40] exp(out)
        out = np.log(e).astype(np.float32)
        lse = np.log(e.astype(np.float64).sum(axis=1)).astype(np.float32)
        lp = out - lse[:, None]
        lps.append(lp)
        outs.append(out)
    return np.concatenate(lps), np.concatenate(outs)


def _bern_prop_host(h, edge_index, theta):
    """Fallback: full Bernstein propagation on host (only if temp != ones)."""
    from math import comb

    n = h.shape[0]
    src = np.asarray(edge_index[0], np.int64)
    dst = np.asarray(edge_index[1], np.int64)
    deg = np.bincount(src, minlength=n).astype(np.float32)
    dis = np.where(deg > 0, 1.0 / np.sqrt(np.maximum(deg, 1.0)), 0.0).astype(
        np.float32
    )

    def anorm(v):
        msg = v[src] * dis[src][:, None]
        out = np.zeros_like(v)
        np.add.at(out, dst, msg)
        return out * dis[:, None]

    K = len(theta) - 1
    tmp = [h]
    for _ in range(K):
        t = tmp[-1]
        tmp.append(t + anorm(t))
    c = np.array([comb(K, j) / 2.0**K for j in range(K + 1)], np.float32)
    acc = np.zeros_like(h)
    for j in range(K, 0, -1):
        s = acc + c[j] * theta[j] * tmp[K - j]
        acc = s - anorm(s)
    return c[0] * theta[0] * tmp[K] + acc


def kernel(x, edge_index, W1, b1, W2, b2, temp):
    from concourse.bass_utils import run_bass_kernel_spmd

    nc = _get_bass()
    in_maps = _in_maps(x, W1, b1, W2, b2)
    res = run_bass_kernel_spmd(nc, in_maps, core_ids=list(range(_NCORES)))
    lp, out = _unshard(res)

    theta = np.maximum(np.asarray(temp, np.float32), 0.0)
    if not np.allclose(theta, 1.0):
        # General-temp path: device computed h; propagate on host, then
        # recompute log_softmax.
        out = _bern_prop_host(out.astype(np.float32), edge_index, theta)
        m = out.max(axis=1, keepdims=True)
        lp = out - (np.log(np.exp(out - m).sum(axis=1, keepdims=True)) + m)
        lp = lp.astype(np.float32)

    return lp, out


# revision 49
# speedup vs baseline: 1.0901x; 1.0102x over previous
"""Trainium2 kernel for nn_BernNet_47364899340878.

Math note (why the device kernel is just the MLP):
  The reference computes  out = sum_{j=0..K} c_j * relu(temp_j) * L^j (2I-L)^{K-j} h
  with c_j = C(K,j)/2^K and h = relu(x@W1+b1)@W2+b2.  The graded inputs pin
  temp = ones (spec fill "ones"), so relu(temp_j) = 1 for all j.  L and
  (2I - L) are commuting polynomials in the normalized adjacency, so the
  binomial theorem gives

      sum_j C(K,j) L^j (2I-L)^{K-j} = (L + 2I - L)^K = (2I)^K = 2^K I,

  i.e. the whole K=10 Bernstein propagation is exactly the identity map and
  out == h.  A non-ones temp (never the case for the graded inputs) falls
  back to a host implementation of the propagation for correctness.

Device kernel: h = relu(x@W1+b1), e = exp(h@W2 + b2), row-sharded over 8
NeuronCores (12500 rows each).  The device ships e (bf16) only; the host
finishes log_softmax exactly: out = ln(e), lse = ln(sum e), logp = out-lse.
The kernel is HBM-bandwidth AND power-envelope bound; traffic per core is
~12.8 MB in + ~1.0 MB out.  Design notes (from perfetto/ntff analysis):
  - every input DMA uses ALL 128 SBUF partitions (HWDGE splits a transfer
    across SDMA engines by partition count; 128 -> all 16 engines).  The
    contraction is host-padded 500 -> 512 = 4 x 128 chunks,
  - ALL of x stays resident in SBUF (100KB of the 208KB/partition):
    persistent group tiles (w1 + block 0 fused in ONE starter DMA, then 12
    x 1000-row pairs), so no pool-recycle gating; pair groups are issued
    ~3 ahead of consumption — a free-running 425 GB/s burst costs PE
    clock (power throttle, HAM duty drops 8/8 -> 4/8) more than it buys,
  - PARTITION-PACKED blocks: per 500-row block, mm1 interleaves two
    4-chunk accumulation groups so rows 0..249 land on PSUM partitions
    0..63 and rows 250..499 on partitions 64..127 (the 64-wide W1
    stationary selects the output partition range).  relu+bias is then
    ONE DVE op [128,250] -> bf16, mm2 is ONE matmul (block-diagonal
    stacked W2 [128, 80] stationary, relu-h moving) -> packed logits^T
    [80,250], and exp is ONE ACT op that reads PSUM and writes the bf16
    output tile with bias=b2 (exact b2 handling).  Halving the DVE/ACT
    op count per block is what lets the epilogue keep up with the PE,
  - the device runs NO reduce / Ln / subtract / copy: the host sums the
    SAME bf16 exp values in fp64 (lse error ~6e-4, far under the bf16
    matmul noise ~5e-3 vs the 2e-2 gate),
  - a memset scratch feeds 10 PE warm-up matmuls + ACT/DVE warm-ups
    emitted before any data lands: starts the HAM clock ramp early and
    pre-loads the ACT function table off the critical path,
  - Exp and Relu are pinned to one ACT table set so the kernel does a
    single table load,
  - output quads [80, 4x250] bf16 (80 partitions -> 16-engine split) ship
    via GpSimd SWDGE so their issue cost never delays ACT's exp queue;
    the 'last' block rides the by-then-idle Sync queue concurrently.
"""

import numpy as np

_N = 100000
_FIN = 500
_FPAD = 512  # contraction padded to 4 chunks x 128 partitions
_HID = 64
_CLS = 40
_NCORES = 8
_RPC = _N // _NCORES  # 12500 rows per core
_P = 128  # contraction partitions per chunk
_KC = 4  # contraction chunks
_BLK = 500  # rows per block
_NBLK = _RPC // _BLK  # 25
_NPAIR = 12  # paired 1000-row groups; block 0 is the 500-row starter
_NQUAD = 6  # output DMAs of 4 blocks each (blocks 0..23)

_CACHE = {}


def _build_bass():
    """Build the per-core Bass program (shared by all 8 cores)."""
    from contextlib import ExitStack

    import concourse.bacc as bacc
    import concourse.mybir as mybir
    import concourse.tile as tile

    fp32 = mybir.dt.float32
    bf16 = mybir.dt.bfloat16
    AF = mybir.ActivationFunctionType
    OP = mybir.AluOpType

    # Bacc (not plain Bass): its compile() runs move_matmul_waits_to_ldweights
    # + generate_event_semaphores, which split excess on_wait entries to meet
    # TRN2's 1-wait-per-instruction constraint that walrus enforces.
    #
    # Table-set pinning: ACT function tables are loaded as named sets and a
    # set switch costs ~1.3-2.7us.  Restricting Exp/Relu to one shared set
    # (keeping every set's positional id intact) makes the whole kernel need
    # exactly one load.
    class _PinnedActBacc(bacc.Bacc):
        def insert_act_table_loads(self):
            import bass_rust as _bass_rust
            from concourse.hw_specs import get_activation_tables

            has_activation = any(
                isinstance(i, mybir.InstActivation)
                for b in self.main_func.blocks
                for i in b.instructions
            )
            if not has_activation:
                return
            shared = {AF.Exp, AF.Ln, AF.Relu}
            tables = []
            for name, fns in get_activation_tables(self.m.arch).items():
                if name != "natural_log_exp_and_others":
                    fns = fns - shared
                tables.append((name, fns))
            _bass_rust.insert_act_table_loads(self, tables)

    nc = _PinnedActBacc()
    # w1 + block-0's two quarter groups fused into ONE tensor/DMA: a single
    # issue + completion, so the first real matmul gates on one transfer
    # that lands ~9.8us (w1 cols 0:256, x0a 256:1256, x0b 1256:2256)
    wx0 = nc.dram_tensor("wx0", [_P, 256 + 2 * _KC * (_BLK // 2)], bf16, kind="ExternalInput")
    xgp = nc.dram_tensor("xgp", [_NPAIR, _P, _KC, 2 * _BLK], bf16, kind="ExternalInput")
    b1 = nc.dram_tensor("b1", [_P, 1], fp32, kind="ExternalInput")
    w2 = nc.dram_tensor("w2", [_P, 2 * _CLS], bf16, kind="ExternalInput")
    b2 = nc.dram_tensor("b2", [2 * _CLS, 1], fp32, kind="ExternalInput")
    # exp(logits)^T quads, partition-PACKED: partitions 0..39 = classes of
    # rows 0..249, partitions 40..79 = classes of rows 250..499.  80
    # partitions -> the HWDGE spreads each quad over all 16 SDMA engines.
    both = nc.dram_tensor(
        "both", [_NQUAD, 2 * _CLS, 4, _BLK // 2], bf16, kind="ExternalOutput"
    )
    last = nc.dram_tensor("last", [2 * _CLS, _BLK // 2], bf16, kind="ExternalOutput")

    xgp_r = xgp.rearrange("pr p kc r -> pr p kc r")
    both_r = both.rearrange("q c k r -> q c k r")

    with tile.TileContext(nc) as tc, ExitStack() as ctx:
        const = ctx.enter_context(tc.tile_pool(name="const", bufs=1))
        xpool = ctx.enter_context(tc.tile_pool(name="xin", bufs=_NPAIR))
        htpool = ctx.enter_context(tc.tile_pool(name="hrelu", bufs=3))
        cpool = ctx.enter_context(tc.tile_pool(name="outs", bufs=2))
        pp1 = ctx.enter_context(tc.tile_pool(name="ps1", bufs=3, space="PSUM"))
        pp2 = ctx.enter_context(tc.tile_pool(name="ps2", bufs=4, space="PSUM"))
        ppw = ctx.enter_context(tc.tile_pool(name="psw", bufs=1, space="PSUM"))

        # Engine warm-ups, emitted BEFORE any DMA so they only gate on a
        # local memset: 10 matmuls keep the PE busy early (the HAM clock
        # ramps on sustained activity, so real blocks start closer to full
        # rate), and the ACT warm-ups trigger the one-time function-table
        # load off the critical path.  More warm-ups burn power credit the
        # governor later claws back — 10 measured best.
        warm_sb = const.tile([_P, 128], bf16)
        nc.vector.memset(warm_sb[:], 0.0)
        warm_ps = ppw.tile([_P, 128], fp32)
        for _ in range(10):
            nc.tensor.matmul(warm_ps[:], warm_sb[:], warm_sb[:])
        warm_o = const.tile([_P, 128], bf16)
        nc.scalar.activation(warm_o[:], warm_sb[:], AF.Relu)
        nc.scalar.activation(warm_o[:], warm_sb[:], AF.Exp)
        nc.vector.tensor_scalar(
            out=warm_o[:], in0=warm_sb[:], scalar1=0.0, scalar2=0.0,
            op0=OP.add, op1=OP.max,
        )

        # Issue order: the fused w1+block-0 tensor first (first real matmul
        # gates on this single transfer), then pair groups 0/1, then the
        # tiny b1/w2/b2 on the ACT queue.  Remaining groups are issued ~3
        # ahead of consumption from stage1.
        wx0_sb = const.tile([_P, 256 + 2 * _KC * (_BLK // 2)], bf16)
        nc.sync.dma_start(wx0_sb[:], wx0[:])

        def w1_ap(kc):
            return wx0_sb[:, kc * _HID : (kc + 1) * _HID]

        def x0a_ap(kc):
            o = 256 + kc * (_BLK // 2)
            return wx0_sb[:, o : o + _BLK // 2]

        def x0b_ap(kc):
            o = 256 + _KC * (_BLK // 2) + kc * (_BLK // 2)
            return wx0_sb[:, o : o + _BLK // 2]

        pair_sb = {}

        def issue_pair(pr):
            t = xpool.tile([_P, _KC, 2 * _BLK], bf16, tag="xt", name=f"xtp{pr}")
            nc.sync.dma_start(t[:], xgp_r[pr])
            pair_sb[pr] = t

        issue_pair(0)
        issue_pair(1)
        issue_pair(2)
        # tiny consts ride the ACT engine's DGE trigger so they never queue
        # behind the x stream on Sync's queue
        b1_sb = const.tile([_P, 1], fp32)
        nc.scalar.dma_start(b1_sb[:], b1[:])
        w2_sb = const.tile([_P, 2 * _CLS], bf16)
        nc.scalar.dma_start(w2_sb[:], w2[:])
        b2_sb = const.tile([2 * _CLS, 1], fp32)
        nc.scalar.dma_start(b2_sb[:], b2[:])

        def mm1_block(srcs):
            # PARTITION-PACKED h^T: rows 0..249 of the block land on PSUM
            # partitions 0..63, rows 250..499 on partitions 64..127, via two
            # interleaved 4-chunk accumulation groups sharing each chunk's
            # 64-wide W1 stationary (one LDWEIGHTS per chunk).  Downstream
            # relu/mm2/exp then each run as ONE op on a 250-col tile —
            # halving the DVE and ACT time per block that otherwise paces
            # the whole pipeline.
            sa, sb = srcs
            h = pp1.tile([_P, _BLK // 2], fp32)
            for kc in range(_KC):
                nc.tensor.matmul(
                    h[:_HID, :], w1_ap(kc), sa(kc),
                    start=(kc == 0), stop=(kc == _KC - 1),
                )
                nc.tensor.matmul(
                    h[_HID:, :], w1_ap(kc), sb(kc),
                    start=(kc == 0), stop=(kc == _KC - 1),
                )
            return h

        def relu_mm2(h):
            # ONE fused bias+relu (DVE) on the packed [128, 250] tile, then
            # ONE matmul: stationary block-diag W2 [128, 80], moving relu-h
            # -> packed logits^T [80, 250].
            ht = htpool.tile([_P, _BLK // 2], bf16)
            nc.vector.tensor_scalar(
                out=ht[:], in0=h[:],
                scalar1=b1_sb[:], scalar2=0.0, op0=OP.add, op1=OP.max,
            )
            o_ps = pp2.tile([2 * _CLS, _BLK // 2], fp32)
            nc.tensor.matmul(o_ps[:], w2_sb[:], ht[:])
            return o_ps

        cmb_last = cpool.tile([2 * _CLS, _BLK // 2], bf16, tag="cl")
        cmb_quad = {}

        def cmb_slot(b):
            if b == _NBLK - 1:
                return cmb_last[:]
            q = b // 4
            if q not in cmb_quad:
                cmb_quad[q] = cpool.tile(
                    [2 * _CLS, 4, _BLK // 2], bf16, tag="cq", name=f"cmbq{q}"
                )
            return cmb_quad[q][:, b % 4]

        ht_ps_of = {}
        o_ps_of = {}

        def stage1(b):
            if b == 0:
                ht_ps_of[b] = mm1_block((x0a_ap, x0b_ap))
            else:
                pr, k = divmod(b - 1, 2)
                if k == 0 and pr + 3 < _NPAIR and pr + 3 not in pair_sb:
                    issue_pair(pr + 3)
                t = pair_sb[pr]
                lo = k * _BLK
                ht_ps_of[b] = mm1_block((
                    lambda kc, t=t, lo=lo: t[:, kc, lo : lo + _BLK // 2],
                    lambda kc, t=t, lo=lo + _BLK // 2: t[:, kc, lo : lo + _BLK // 2],
                ))

        def stage23(b):
            o_ps_of[b] = relu_mm2(ht_ps_of.pop(b))

        def stage4(b):
            # exp(logits + b2) straight from PSUM into the bf16 output slot
            nc.scalar.activation(cmb_slot(b), o_ps_of.pop(b)[:], AF.Exp, bias=b2_sb[:])
            if b == _NBLK - 1:
                # Sync queue is idle once inputs are done; the final quad
                # (GpSimd) and the last block then ship concurrently
                nc.sync.dma_start(last[:], cmb_last[:])
            elif b % 4 == 3:
                # one DMA per quad via GpSimd SWDGE — the engine is idle, so
                # the ~1us descriptor-issue cost never delays ACT's exp queue
                nc.gpsimd.dma_start(both_r[b // 4], cmb_quad.pop(b // 4)[:])

        seq = list(range(_NBLK))
        for idx, b in enumerate(seq):
            stage1(b)
            if idx >= 1:
                stage23(seq[idx - 1])
            if idx >= 2:
                stage4(seq[idx - 2])
        stage23(seq[-1])
        stage4(seq[-2])
        stage4(seq[-1])

    nc.finalize()
    return nc


def _get_bass():
    if "nc" not in _CACHE:
        _CACHE["nc"] = _build_bass()
    return _CACHE["nc"]


def _host_prep(x, W1, b1, W2, b2):
    """Weights/bias in device layout (bf16, FWL/DMA-padded)."""
    import ml_dtypes

    bf = ml_dtypes.bfloat16
    x = np.asarray(x, np.float32)
    x_bf = np.zeros((x.shape[0], _FPAD), bf)
    x_bf[:, :_FIN] = x.astype(bf)  # [N, 512]
    W1b = np.zeros((_FPAD, _HID), bf)
    W1b[:_FIN] = np.asarray(W1, np.float32).astype(bf)
    # feature f = kc*128 + p  ->  w1p[p, kc, m]
    w1p = np.ascontiguousarray(W1b.reshape(_KC, _P, _HID).transpose(1, 0, 2))
    # packed-partition layout: bias/W2/b2 duplicated for both row halves
    b1f = np.asarray(b1, np.float32).reshape(_HID)
    b1a = np.ascontiguousarray(np.concatenate([b1f, b1f]).reshape(_P, 1))
    w2f = np.asarray(W2, np.float32).astype(bf)
    w2a = np.zeros((_P, 2 * _CLS), bf)
    w2a[:_HID, :_CLS] = w2f
    w2a[_HID:, _CLS:] = w2f
    b2f = np.asarray(b2, np.float32).reshape(_CLS)
    b2a = np.ascontiguousarray(np.concatenate([b2f, b2f]).reshape(2 * _CLS, 1))
    return x_bf, w1p, b1a, w2a, b2a


def _core_x(x_bf, c):
    """Per-core inputs: 500-row starter group + 12 paired 1000-row groups."""
    xs = x_bf[c * _RPC : (c + 1) * _RPC]  # [12500, 512]
    # starter: rows 0..499 as two 250-row quarter groups; feature = kc*128+p
    x0 = (
        xs[:_BLK].reshape(2, _BLK // 2, _KC, _P).transpose(0, 3, 2, 1)
        .reshape(2, _P, _KC * (_BLK // 2))
    )
    # pairs: row = 500 + pr*1000 + r
    xp = np.ascontiguousarray(
        xs[_BLK:].reshape(_NPAIR, 2 * _BLK, _KC, _P).transpose(0, 3, 2, 1)
    )
    return x0, xp


def _in_maps(x, W1, b1, W2, b2):
    x_bf, w1p, b1a, w2a, b2a = _host_prep(x, W1, b1, W2, b2)
    maps = []
    for c in range(_NCORES):
        x0, xp = _core_x(x_bf, c)
        wx0 = np.concatenate(
            [w1p.reshape(_P, _KC * _HID), x0[0], x0[1]], axis=1
        )
        maps.append(
            {"wx0": wx0, "xgp": xp, "b1": b1a, "w2": w2a, "b2": b2a}
        )
    return maps


def _unshard(res):
    """Device ships bf16 exp(out)^T; host finishes log_softmax via ln/sum."""
    outs = []
    lps = []
    for c in range(_NCORES):
        a = np.asarray(res.results[c]["both"]).astype(np.float32)
        l = np.asarray(res.results[c]["last"]).astype(np.float32)
        # a[q, half*cls, kq, r] -> rows (q, kq, half, r)
        la = (
            a.reshape(_NQUAD, 2, _CLS, 4, _BLK // 2)
            .transpose(0, 3, 1, 4, 2)
            .reshape(_NQUAD * 4 * _BLK, _CLS)
        )
        ll = l.reshape(2, _CLS, _BLK // 2).transpose(0, 2, 1).reshape(_BLK, _CLS)
        e = np.concatenate([la, ll])  # [12500, 40] exp(out)
        out = np.log(e).astype(np.float32)
        lse = np.log(e.astype(np.float64).sum(axis=1)).astype(np.float32)
        lp = out - lse[:, None]
        lps.append(lp)
        outs.append(out)
    return np.concatenate(lps), np.concatenate(outs)


def _bern_prop_host(h, edge_index, theta):
    """Fallback: full Bernstein propagation on host (only if temp != ones)."""
    from math import comb

    n = h.shape[0]
    src = np.asarray(edge_index[0], np.int64)
    dst = np.asarray(edge_index[1], np.int64)
    deg = np.bincount(src, minlength=n).astype(np.float32)
    dis = np.where(deg > 0, 1.0 / np.sqrt(np.maximum(deg, 1.0)), 0.0).astype(
        np.float32
    )

    def anorm(v):
        msg = v[src] * dis[src][:, None]
        out = np.zeros_like(v)
        np.add.at(out, dst, msg)
        return out * dis[:, None]

    K = len(theta) - 1
    tmp = [h]
    for _ in range(K):
        t = tmp[-1]
        tmp.append(t + anorm(t))
    c = np.array([comb(K, j) / 2.0**K for j in range(K + 1)], np.float32)
    acc = np.zeros_like(h)
    for j in range(K, 0, -1):
        s = acc + c[j] * theta[j] * tmp[K - j]
        acc = s - anorm(s)
    return c[0] * theta[0] * tmp[K] + acc


def kernel(x, edge_index, W1, b1, W2, b2, temp):
    from concourse.bass_utils import run_bass_kernel_spmd

    nc = _get_bass()
    in_maps = _in_maps(x, W1, b1, W2, b2)
    res = run_bass_kernel_spmd(nc, in_maps, core_ids=list(range(_NCORES)))
    lp, out = _unshard(res)

    theta = np.maximum(np.asarray(temp, np.float32), 0.0)
    if not np.allclose(theta, 1.0):
        # General-temp path: device computed h; propagate on host, then
        # recompute log_softmax.
        out = _bern_prop_host(out.astype(np.float32), edge_index, theta)
        m = out.max(axis=1, keepdims=True)
        lp = out - (np.log(np.exp(out - m).sum(axis=1, keepdims=True)) + m)
        lp = lp.astype(np.float32)

    return lp, out
